# revision 8
# baseline (speedup 1.0000x reference)
"""BiDAF-style co-attention (memory_len=2) Trainium2 Bass kernel.

Full inputs:
  input     [8, 4096, 1024] f32
  memory    [8, 2, 1024]    f32
  w_input   [1024] f32, w_memory [1024] f32, dot_scale [1024] f32
Output:
  concat([input, output_one, input*output_one, output_two*output_one], -1)
  -> [8, 4096, 4096] f32

Sharding: data-parallel over batch; core b gets batch b (8 cores).

Math (per batch), 2-way softmax as a sigmoid:
  v_m   = w_input + dot_scale * mem_m            (d-vector, m=0,1)
  c_m   = mem_m . w_memory                       (scalar)
  delta[l] = input[l].(v1-v0);  u0[l] = input[l].v0   (two DVE dot passes)
  w1[l] = sigmoid(delta + cdiff)                 (ACT, bias=cdiff)
  output_one[l] = mem0 + w1[l]*(mem1-mem0)       (PE rank-2, stationary (1,w1),
                                                  rhs rows (mem0, mem1-mem0))
  a[l]  = max(u0, u0+delta+cdiff); wexp = exp(a)
  output_two = (sum_l wexp[l]*input[l]) / sum wexp   (PE matvec, PSUM accum)
  comp3[l] = o2n*output_one[l] = q0 + w1[l]*qd   (PE, same stationary,
                                                  rhs rows (o2n*mem0, o2n*memd))
PE matmuls run in bf16 (1 cyc/row); setup/broadcasts stay fp32 exact.
"""

import numpy as np

B, L, D = 8, 4096, 1024
T = L // 128  # 32 row-tiles of 128
G = 8         # tiles per group (batched small stats ops)

_CACHE = {}

# stats column layout ([128, NSTAT] f32), blocks of 32 (col t = tile t)
DL = 0       # delta = in . (v1-v0)
U0 = 32      # u0 = in . v0
Z1 = 64      # z1u = delta + cdiff + u0
AA = 96      # a = max(z1u, u0)
WE = 128     # wexp = exp(a)
W0 = 160     # all-ones block (memset)  -> stationary pair (W0+t, W1+t)
W1 = 192     # w1 = sigmoid(delta + cdiff)
OC = 224     # ones column
CD, SE, ST, SR = 225, 226, 227, 228
NSTAT = 232


def _build():
    import concourse.bacc as bacc
    import concourse.bass as bass
    import concourse.tile as tile
    from concourse import mybir
    from concourse.masks import make_identity

    f32 = mybir.dt.float32
    bf16 = mybir.dt.bfloat16
    ALU = mybir.AluOpType
    ACT = mybir.ActivationFunctionType

    nc = bacc.Bacc("TRN2", target_bir_lowering=False, debug=False)

    inp = nc.dram_tensor("input", [L, D], f32, kind="ExternalInput").ap()
    mem = nc.dram_tensor("memory", [2, D], f32, kind="ExternalInput").ap()
    w_in = nc.dram_tensor("w_input", [D], f32, kind="ExternalInput").ap()
    w_mem = nc.dram_tensor("w_memory", [D], f32, kind="ExternalInput").ap()
    d_sc = nc.dram_tensor("dot_scale", [D], f32, kind="ExternalInput").ap()
    out = nc.dram_tensor("out", [L, 4 * D], f32, kind="ExternalOutput").ap()

    def bc(src_ap, n_part, n_free):
        # broadcast-read AP: n_part partitions each reading the same n_free
        # contiguous elements at src_ap's offset (DMA-only pattern)
        return bass.AP(src_ap.tensor, src_ap.offset, [[0, n_part], [1, n_free]])

    ts = bass.ts

    with tile.TileContext(nc) as tc:
        with (
            tc.tile_pool(name="consts", bufs=1) as consts,
            tc.tile_pool(name="setup2d", bufs=3) as setup2d,
            tc.tile_pool(name="setup1d", bufs=2) as setup1d,
            tc.tile_pool(name="inp_pool", bufs=12) as inp_pool,
            tc.tile_pool(name="inbf_pool", bufs=12) as inbf_pool,
            tc.tile_pool(name="scratch", bufs=2) as scratch,
            tc.tile_pool(name="stage12", bufs=4) as stage12p,
            tc.tile_pool(name="stage3", bufs=6) as stage3p,
            tc.tile_pool(name="o1ps", bufs=2, space="PSUM") as o1psp,
            tc.tile_pool(name="o2ps", bufs=1, space="PSUM") as o2psp,
            tc.tile_pool(name="wstps", bufs=2, space="PSUM") as wstpsp,
        ):
            # ---------------- setup ----------------
            mem_sb = consts.tile([2, D], f32)
            nc.sync.dma_start(out=mem_sb, in_=mem)
            stats = consts.tile([128, NSTAT], f32)
            identity = consts.tile([128, 128], f32)
            make_identity(nc, identity)
            nc.vector.memset(stats[:, OC : OC + 1], 1.0)
            nc.vector.memset(stats[:, W0 : W0 + T], 1.0)
            # est: per-tile transposed (1, w1) stationaries, col-block t
            est = consts.tile([2, T * 128], bf16)
            # strided pair view: pair_view[:, t, :] = cols (W0+t, W1+t)
            pair_view = stats[:, W0 : W0 + 2 * T].rearrange("p (a b) -> p b a", a=2)

            # small constant stationaries (engine ops must start at partition
            # 0, so build 2-partition constants from identity slices)
            p10 = identity[0:2, 0:1]            # col (x0) = [1, 0]
            pm1 = consts.tile([2, 1], f32)      # col (x1 - x0) = [-1, 1]
            nc.vector.tensor_tensor(
                out=pm1, in0=identity[0:2, 1:2], in1=identity[0:2, 0:1],
                op=ALU.subtract,
            )
            w22 = consts.tile([2, 2], f32)      # [[1,-1],[0,1]]: cols (x0, x1-x0)
            nc.vector.tensor_copy(w22, identity[0:2, 0:2])
            nc.vector.tensor_tensor(
                out=w22[:, 1:2], in0=w22[:, 1:2], in1=identity[0:2, 0:1],
                op=ALU.subtract,
            )
            ones1 = consts.tile([1, 128], f32)  # broadcast to 128 partitions
            nc.vector.memset(ones1, 1.0)
            ones2 = consts.tile([1, 2], f32)    # broadcast to 2 partitions
            nc.vector.memset(ones2, 1.0)

            ds_b = setup2d.tile([2, D], f32, tag="s2d")
            nc.sync.dma_start(out=ds_b, in_=bc(d_sc, 2, D))
            win_b = setup2d.tile([2, D], f32, tag="s2d")
            nc.sync.dma_start(out=win_b, in_=bc(w_in, 2, D))
            # vcat = mem*ds + w_in  (rows: v0, v1)
            vcat = setup2d.tile([2, D], f32, tag="s2d")
            nc.vector.tensor_tensor(out=vcat, in0=mem_sb, in1=ds_b, op=ALU.mult)
            nc.vector.tensor_tensor(out=vcat, in0=vcat, in1=win_b, op=ALU.add)

            # m0d = (mem0, mem1-mem0) rows via PE; v0/vdiff as partition-0 rows
            m0d = consts.tile([2, D], f32)
            m0db = consts.tile([2, D], bf16)
            v0row = setup1d.tile([1, D], f32, tag="s1d")
            vdrow = setup1d.tile([1, D], f32, tag="s1d")
            for h in range(2):
                md_ps = wstpsp.tile([2, 512], f32, tag="wst")
                nc.tensor.matmul(
                    md_ps, lhsT=w22, rhs=mem_sb[:, ts(h, 512)], start=True, stop=True
                )
                nc.scalar.copy(m0d[:, ts(h, 512)], md_ps)
                nc.scalar.copy(m0db[:, ts(h, 512)], md_ps)
                v0_ps = wstpsp.tile([1, 512], f32, tag="wst")
                nc.tensor.matmul(
                    v0_ps, lhsT=p10, rhs=vcat[:, ts(h, 512)], start=True, stop=True
                )
                nc.scalar.copy(v0row[:, ts(h, 512)], v0_ps)
                vd_ps = wstpsp.tile([1, 512], f32, tag="wst")
                nc.tensor.matmul(
                    vd_ps, lhsT=pm1, rhs=vcat[:, ts(h, 512)], start=True, stop=True
                )
                nc.scalar.copy(vdrow[:, ts(h, 512)], vd_ps)

            # broadcast v0 / vdiff rows to 128 partitions via PE (exact fp32)
            v0b = consts.tile([128, D], bf16)
            vdb = consts.tile([128, D], bf16)
            for h in range(2):
                b_ps = o1psp.tile([128, 512], f32, tag="o1")
                nc.tensor.matmul(
                    b_ps, lhsT=ones1, rhs=v0row[:, ts(h, 512)], start=True, stop=True
                )
                nc.scalar.copy(v0b[:, ts(h, 512)], b_ps)
                b_ps2 = o1psp.tile([128, 512], f32, tag="o1")
                nc.tensor.matmul(
                    b_ps2, lhsT=ones1, rhs=vdrow[:, ts(h, 512)], start=True, stop=True
                )
                nc.scalar.copy(vdb[:, ts(h, 512)], b_ps2)

            # memdot = (mem * w_memory).sum(-1) -> [2,1]; cdc = bcast(c1-c0)
            wmem_b = setup2d.tile([2, D], f32, tag="s2d")
            nc.sync.dma_start(out=wmem_b, in_=bc(w_mem, 2, D))
            sc2 = setup2d.tile([2, D], f32, tag="s2d")
            md_col = setup1d.tile([2, 1], f32, tag="s1s")
            nc.vector.scalar_tensor_tensor(
                out=sc2, in0=mem_sb, scalar=1.0, in1=wmem_b,
                op0=ALU.mult, op1=ALU.mult,
                accum_out=md_col,
            )
            cd_ps = wstpsp.tile([1, 1], f32, tag="wst")
            nc.tensor.matmul(cd_ps, lhsT=pm1, rhs=md_col, start=True, stop=True)
            cd_sb = setup1d.tile([1, 1], f32, tag="s1s")
            nc.scalar.copy(cd_sb, cd_ps)
            cdc_ps = wstpsp.tile([128, 1], f32, tag="wst")
            nc.tensor.matmul(cdc_ps, lhsT=ones1, rhs=cd_sb, start=True, stop=True)
            nc.scalar.copy(stats[:, CD : CD + 1], cdc_ps)

            webf = consts.tile([128, T], bf16)

            cdc = stats[:, CD : CD + 1]

            # persistent PSUM accumulator for output_two partials [1, D]
            o2_ps = o2psp.tile([1, D], f32)

            # ---------------- main pass ----------------
            for g in range(0, T, G):
                in_ts = {}
                in_bfs = {}
                # per-tile: load + two fused att dots (DVE)
                for t in range(g, g + G):
                    in_t = inp_pool.tile([128, D], f32, tag="in_t")
                    in_ts[t] = in_t
                    nc.sync.dma_start(out=in_t, in_=inp[ts(t, 128), :])
                    # comp0: passthrough copy of input
                    nc.gpsimd.dma_start(out=out[ts(t, 128), 0:D], in_=in_t)
                    in_bf = inbf_pool.tile([128, D], bf16, tag="in_bf")
                    in_bfs[t] = in_bf
                    nc.gpsimd.tensor_copy(in_bf, in_t)
                    sc_t = scratch.tile([128, D], bf16, tag="ttr")
                    nc.vector.scalar_tensor_tensor(
                        out=sc_t, in0=in_bf, scalar=1.0, in1=vdb,
                        op0=ALU.mult, op1=ALU.mult,
                        accum_out=stats[:, DL + t : DL + t + 1],
                    )
                    sc_t2 = scratch.tile([128, D], bf16, tag="ttr")
                    nc.vector.scalar_tensor_tensor(
                        out=sc_t2, in0=in_bf, scalar=1.0, in1=v0b,
                        op0=ALU.mult, op1=ALU.mult,
                        accum_out=stats[:, U0 + t : U0 + t + 1],
                    )

                # batched group stats ([128, G] blocks)
                dlb = stats[:, DL + g : DL + g + G]
                u0b = stats[:, U0 + g : U0 + g + G]
                z1b = stats[:, Z1 + g : Z1 + g + G]
                aab = stats[:, AA + g : AA + g + G]
                web = stats[:, WE + g : WE + g + G]
                w1b = stats[:, W1 + g : W1 + g + G]
                # z1u = (delta + cdiff) + u0
                nc.vector.scalar_tensor_tensor(
                    out=z1b, in0=dlb, scalar=cdc, in1=u0b,
                    op0=ALU.add, op1=ALU.add,
                )
                # a = max(z1u, u0)
                nc.vector.tensor_tensor(out=aab, in0=z1b, in1=u0b, op=ALU.max)
                nc.scalar.activation(out=web, in_=aab, func=ACT.Exp)
                nc.scalar.copy(webf[:, g : g + G], web)
                # w1 = sigmoid(delta + cdiff)
                nc.scalar.activation(out=w1b, in_=dlb, func=ACT.Sigmoid, bias=cdc)

                # per-tile: PE outer products + comp2 + output_two accum
                for t in range(g, g + G):
                    in_t = in_ts[t]
                    # stationary [2,128] = transpose of the (1, w1_t) pair
                    wst_ps = wstpsp.tile([2, 128], f32, tag="wst")
                    nc.tensor.transpose(wst_ps, pair_view[:, t, :], identity)
                    nc.scalar.copy(est[:, ts(t, 128)], wst_ps)

                    # output_one = 1*mem0 + w1*(mem1-mem0)
                    o1_ps = o1psp.tile([128, D], f32, tag="o1")
                    for h in range(2):
                        nc.tensor.matmul(
                            o1_ps[:, ts(h, 512)],
                            lhsT=est[:, ts(t, 128)],
                            rhs=m0db[:, ts(h, 512)],
                            start=True,
                            stop=True,
                        )
                    st12 = stage12p.tile([128, 2 * D], f32, tag="s12")
                    nc.scalar.activation(out=st12[:, 0:D], in_=o1_ps, func=ACT.Copy)
                    # comp2 = input * output_one
                    nc.vector.tensor_tensor(
                        out=st12[:, D : 2 * D], in0=in_t, in1=o1_ps, op=ALU.mult
                    )
                    nc.scalar.dma_start(out=out[ts(t, 128), D : 3 * D], in_=st12)
                    # output_two partials: o2_ps += wexp_t^T @ in_t (PE accum)
                    in_bf = in_bfs[t]
                    for h in range(2):
                        nc.tensor.matmul(
                            o2_ps[0:1, ts(h, 512)],
                            lhsT=webf[:, t : t + 1],
                            rhs=in_bf[:, ts(h, 512)],
                            start=(t == 0),
                            stop=(t == T - 1),
                        )

            # ---------------- output_two normalize + q rows ----------------
            nc.vector.tensor_reduce(
                out=stats[:, SE : SE + 1], in_=stats[:, WE : WE + T],
                axis=mybir.AxisListType.X, op=ALU.add,
            )
            stot_ps = wstpsp.tile([1, 1], f32, tag="wst")
            nc.tensor.matmul(
                stot_ps, lhsT=stats[:, SE : SE + 1], rhs=stats[:, OC : OC + 1],
                start=True, stop=True,
            )
            nc.scalar.copy(stats[0:1, ST : ST + 1], stot_ps)
            nc.vector.reciprocal(stats[0:1, SR : SR + 1], stats[0:1, ST : ST + 1])

            # o2n = output_two (normalized) on partition 0
            o2n_sb = setup1d.tile([1, D], f32, tag="s1d")
            nc.scalar.activation(
                out=o2n_sb, in_=o2_ps, func=ACT.Copy,
                scale=stats[0:1, SR : SR + 1],
            )
            # broadcast o2n to 2 partitions, qcat = o2n * (mem0, memd)
            o2nc = setup2d.tile([2, D], f32, tag="s2d")
            for h in range(2):
                q_ps = wstpsp.tile([2, 512], f32, tag="wst")
                nc.tensor.matmul(
                    q_ps, lhsT=ones2, rhs=o2n_sb[:, ts(h, 512)], start=True, stop=True
                )
                nc.scalar.copy(o2nc[:, ts(h, 512)], q_ps)
            qcat = consts.tile([2, D], bf16)
            nc.vector.tensor_tensor(out=qcat, in0=o2nc, in1=m0d, op=ALU.mult)

            # ---------------- comp3 pass (PE) --------------------------
            for t in range(T):
                c3_ps = o1psp.tile([128, D], f32, tag="o1")
                for h in range(2):
                    nc.tensor.matmul(
                        c3_ps[:, ts(h, 512)],
                        lhsT=est[:, ts(t, 128)],
                        rhs=qcat[:, ts(h, 512)],
                        start=True,
                        stop=True,
                    )
                st3 = stage3p.tile([128, D], f32, tag="s3")
                nc.scalar.copy(st3, c3_ps)
                nc.gpsimd.dma_start(out=out[ts(t, 128), 3 * D : 4 * D], in_=st3)

    nc.compile()
    return nc


def _get_nc():
    if "nc" not in _CACHE:
        _CACHE["nc"] = _build()
    return _CACHE["nc"]


def kernel(input, memory, w_input, w_memory, dot_scale):
    from concourse.bass_utils import run_bass_kernel_spmd

    nc = _get_nc()
    input = np.ascontiguousarray(input, dtype=np.float32)
    memory = np.ascontiguousarray(memory, dtype=np.float32)
    w_input = np.ascontiguousarray(w_input, dtype=np.float32)
    w_memory = np.ascontiguousarray(w_memory, dtype=np.float32)
    dot_scale = np.ascontiguousarray(dot_scale, dtype=np.float32)
    in_maps = [
        {
            "input": input[b],
            "memory": memory[b],
            "w_input": w_input,
            "w_memory": w_memory,
            "dot_scale": dot_scale,
        }
        for b in range(B)
    ]
    res = run_bass_kernel_spmd(nc, in_maps, core_ids=list(range(B)))
    return np.stack([res.results[b]["out"] for b in range(B)], axis=0)


# revision 9
# speedup vs baseline: 1.1923x; 1.1923x over previous
"""BiDAF-style co-attention (memory_len=2) Trainium2 Bass kernel.

Full inputs:
  input     [8, 4096, 1024] f32
  memory    [8, 2, 1024]    f32
  w_input   [1024] f32, w_memory [1024] f32, dot_scale [1024] f32
Output:
  concat([input, output_one, input*output_one, output_two*output_one], -1)
  -> [8, 4096, 4096] f32

Sharding: data-parallel over batch; core b gets batch b (8 cores).

Math (per batch), 2-way softmax as a sigmoid:
  v_m   = w_input + dot_scale * mem_m            (d-vector, m=0,1)
  c_m   = mem_m . w_memory                       (scalar)
  delta[l] = input[l].(v1-v0);  u0[l] = input[l].v0   (two DVE dot passes)
  w1[l] = sigmoid(delta + cdiff)                 (ACT, bias=cdiff)
  output_one[l] = mem0 + w1[l]*(mem1-mem0)       (PE rank-2, stationary (1,w1),
                                                  rhs rows (mem0, mem1-mem0))
  a[l]  = max(u0, u0+delta+cdiff); wexp = exp(a)
  output_two = (sum_l wexp[l]*input[l]) / sum wexp   (PE matvec, PSUM accum)
  comp3[l] = o2n*output_one[l] = q0 + w1[l]*qd   (PE, same stationary,
                                                  rhs rows (o2n*mem0, o2n*memd))
PE matmuls run in bf16 (1 cyc/row); setup/broadcasts stay fp32 exact.
"""

import numpy as np

B, L, D = 8, 4096, 1024
T = L // 128  # 32 row-tiles of 128
G = 8         # tiles per group (batched small stats ops)

_CACHE = {}

# stats column layout ([128, NSTAT] f32), blocks of 32 (col t = tile t)
DL = 0       # delta = in . (v1-v0)
U0 = 32      # u0 = in . v0
Z1 = 64      # z1u = delta + cdiff + u0
AA = 96      # a = max(z1u, u0)
WE = 128     # wexp = exp(a)
W0 = 160     # all-ones block (memset)  -> stationary pair (W0+t, W1+t)
W1 = 192     # w1 = sigmoid(delta + cdiff)
OC = 224     # ones column
CD, SE, ST, SR = 225, 226, 227, 228
NSTAT = 232


def _build():
    import concourse.bacc as bacc
    import concourse.bass as bass
    import concourse.tile as tile
    from concourse import mybir
    from concourse.masks import make_identity

    f32 = mybir.dt.float32
    bf16 = mybir.dt.bfloat16
    ALU = mybir.AluOpType
    ACT = mybir.ActivationFunctionType

    nc = bacc.Bacc("TRN2", target_bir_lowering=False, debug=False)

    inp = nc.dram_tensor("input", [L, D], f32, kind="ExternalInput").ap()
    mem = nc.dram_tensor("memory", [2, D], f32, kind="ExternalInput").ap()
    w_in = nc.dram_tensor("w_input", [D], f32, kind="ExternalInput").ap()
    w_mem = nc.dram_tensor("w_memory", [D], f32, kind="ExternalInput").ap()
    d_sc = nc.dram_tensor("dot_scale", [D], f32, kind="ExternalInput").ap()
    out = nc.dram_tensor("out", [L, 4 * D], f32, kind="ExternalOutput").ap()

    def bc(src_ap, n_part, n_free):
        # broadcast-read AP: n_part partitions each reading the same n_free
        # contiguous elements at src_ap's offset (DMA-only pattern)
        return bass.AP(src_ap.tensor, src_ap.offset, [[0, n_part], [1, n_free]])

    ts = bass.ts

    with tile.TileContext(nc) as tc:
        with (
            tc.tile_pool(name="consts", bufs=1) as consts,
            tc.tile_pool(name="setup2d", bufs=3) as setup2d,
            tc.tile_pool(name="setup1d", bufs=2) as setup1d,
            tc.tile_pool(name="inp_pool", bufs=12) as inp_pool,
            tc.tile_pool(name="inbf_pool", bufs=12) as inbf_pool,
            tc.tile_pool(name="scratch", bufs=2) as scratch,
            tc.tile_pool(name="stage12", bufs=4) as stage12p,
            tc.tile_pool(name="stage3", bufs=6) as stage3p,
            tc.tile_pool(name="o1ps", bufs=2, space="PSUM") as o1psp,
            tc.tile_pool(name="o2ps", bufs=1, space="PSUM") as o2psp,
            tc.tile_pool(name="wstps", bufs=2, space="PSUM") as wstpsp,
        ):
            # ---------------- setup ----------------
            mem_sb = consts.tile([2, D], f32)
            nc.sync.dma_start(out=mem_sb, in_=mem)
            stats = consts.tile([128, NSTAT], f32)
            identity = consts.tile([128, 128], f32)
            make_identity(nc, identity)
            nc.vector.memset(stats[:, OC : OC + 1], 1.0)
            nc.vector.memset(stats[:, W0 : W0 + T], 1.0)
            # est: per-tile transposed (1, w1) stationaries, col-block t
            est = consts.tile([2, T * 128], bf16)
            # strided pair view: pair_view[:, t, :] = cols (W0+t, W1+t)
            pair_view = stats[:, W0 : W0 + 2 * T].rearrange("p (a b) -> p b a", a=2)

            # small constant stationaries (engine ops must start at partition
            # 0, so build 2-partition constants from identity slices)
            p10 = identity[0:2, 0:1]            # col (x0) = [1, 0]
            pm1 = consts.tile([2, 1], f32)      # col (x1 - x0) = [-1, 1]
            nc.vector.tensor_tensor(
                out=pm1, in0=identity[0:2, 1:2], in1=identity[0:2, 0:1],
                op=ALU.subtract,
            )
            w22 = consts.tile([2, 2], f32)      # [[1,-1],[0,1]]: cols (x0, x1-x0)
            nc.vector.tensor_copy(w22, identity[0:2, 0:2])
            nc.vector.tensor_tensor(
                out=w22[:, 1:2], in0=w22[:, 1:2], in1=identity[0:2, 0:1],
                op=ALU.subtract,
            )
            ones1 = consts.tile([1, 128], f32)  # broadcast to 128 partitions
            nc.vector.memset(ones1, 1.0)
            ones2 = consts.tile([1, 2], f32)    # broadcast to 2 partitions
            nc.vector.memset(ones2, 1.0)

            ds_b = setup2d.tile([2, D], f32, tag="s2d")
            nc.sync.dma_start(out=ds_b, in_=bc(d_sc, 2, D))
            win_b = setup2d.tile([2, D], f32, tag="s2d")
            nc.sync.dma_start(out=win_b, in_=bc(w_in, 2, D))
            # vcat = mem*ds + w_in  (rows: v0, v1)
            vcat = setup2d.tile([2, D], f32, tag="s2d")
            nc.vector.tensor_tensor(out=vcat, in0=mem_sb, in1=ds_b, op=ALU.mult)
            nc.vector.tensor_tensor(out=vcat, in0=vcat, in1=win_b, op=ALU.add)

            # m0d = (mem0, mem1-mem0) rows via PE; v0/vdiff as partition-0 rows
            m0d = consts.tile([2, D], f32)
            m0db = consts.tile([2, D], bf16)
            v0row = setup1d.tile([1, D], f32, tag="s1d")
            vdrow = setup1d.tile([1, D], f32, tag="s1d")
            for h in range(2):
                md_ps = wstpsp.tile([2, 512], f32, tag="wst")
                nc.tensor.matmul(
                    md_ps, lhsT=w22, rhs=mem_sb[:, ts(h, 512)], start=True, stop=True
                )
                nc.scalar.copy(m0d[:, ts(h, 512)], md_ps)
                nc.scalar.copy(m0db[:, ts(h, 512)], md_ps)
                v0_ps = wstpsp.tile([1, 512], f32, tag="wst")
                nc.tensor.matmul(
                    v0_ps, lhsT=p10, rhs=vcat[:, ts(h, 512)], start=True, stop=True
                )
                nc.scalar.copy(v0row[:, ts(h, 512)], v0_ps)
                vd_ps = wstpsp.tile([1, 512], f32, tag="wst")
                nc.tensor.matmul(
                    vd_ps, lhsT=pm1, rhs=vcat[:, ts(h, 512)], start=True, stop=True
                )
                nc.scalar.copy(vdrow[:, ts(h, 512)], vd_ps)

            # broadcast v0 / vdiff rows to 128 partitions via PE (exact fp32)
            v0b = consts.tile([128, D], bf16)
            vdb = consts.tile([128, D], bf16)
            for h in range(2):
                b_ps = o1psp.tile([128, 512], f32, tag="o1")
                nc.tensor.matmul(
                    b_ps, lhsT=ones1, rhs=v0row[:, ts(h, 512)], start=True, stop=True
                )
                nc.scalar.copy(v0b[:, ts(h, 512)], b_ps)
                b_ps2 = o1psp.tile([128, 512], f32, tag="o1")
                nc.tensor.matmul(
                    b_ps2, lhsT=ones1, rhs=vdrow[:, ts(h, 512)], start=True, stop=True
                )
                nc.scalar.copy(vdb[:, ts(h, 512)], b_ps2)

            # memdot = (mem * w_memory).sum(-1) -> [2,1]; cdc = bcast(c1-c0)
            wmem_b = setup2d.tile([2, D], f32, tag="s2d")
            nc.sync.dma_start(out=wmem_b, in_=bc(w_mem, 2, D))
            sc2 = setup2d.tile([2, D], f32, tag="s2d")
            md_col = setup1d.tile([2, 1], f32, tag="s1s")
            nc.vector.scalar_tensor_tensor(
                out=sc2, in0=mem_sb, scalar=1.0, in1=wmem_b,
                op0=ALU.mult, op1=ALU.mult,
                accum_out=md_col,
            )
            cd_ps = wstpsp.tile([1, 1], f32, tag="wst")
            nc.tensor.matmul(cd_ps, lhsT=pm1, rhs=md_col, start=True, stop=True)
            cd_sb = setup1d.tile([1, 1], f32, tag="s1s")
            nc.scalar.copy(cd_sb, cd_ps)
            cdc_ps = wstpsp.tile([128, 1], f32, tag="wst")
            nc.tensor.matmul(cdc_ps, lhsT=ones1, rhs=cd_sb, start=True, stop=True)
            nc.scalar.copy(stats[:, CD : CD + 1], cdc_ps)

            webf = consts.tile([128, T], bf16)

            cdc = stats[:, CD : CD + 1]

            # persistent PSUM accumulator for output_two partials [1, D]
            o2_ps = o2psp.tile([1, D], f32)

            # ---------------- main pass ----------------
            for g in range(0, T, G):
                in_ts = {}
                in_bfs = {}
                # per-tile: load + two fused att dots (DVE)
                for t in range(g, g + G):
                    in_t = inp_pool.tile([128, D], f32, tag="in_t")
                    in_ts[t] = in_t
                    nc.sync.dma_start(out=in_t, in_=inp[ts(t, 128), :])
                    # comp0: passthrough copy of input
                    nc.gpsimd.dma_start(out=out[ts(t, 128), 0:D], in_=in_t)
                    in_bf = inbf_pool.tile([128, D], bf16, tag="in_bf")
                    in_bfs[t] = in_bf
                    nc.vector.tensor_copy(in_bf, in_t)
                    sc_t = scratch.tile([128, D], bf16, tag="ttr")
                    nc.vector.scalar_tensor_tensor(
                        out=sc_t, in0=in_bf, scalar=1.0, in1=vdb,
                        op0=ALU.mult, op1=ALU.mult,
                        accum_out=stats[:, DL + t : DL + t + 1],
                    )
                    sc_t2 = scratch.tile([128, D], bf16, tag="ttr")
                    nc.vector.scalar_tensor_tensor(
                        out=sc_t2, in0=in_bf, scalar=1.0, in1=v0b,
                        op0=ALU.mult, op1=ALU.mult,
                        accum_out=stats[:, U0 + t : U0 + t + 1],
                    )

                # batched group stats ([128, G] blocks)
                dlb = stats[:, DL + g : DL + g + G]
                u0b = stats[:, U0 + g : U0 + g + G]
                z1b = stats[:, Z1 + g : Z1 + g + G]
                aab = stats[:, AA + g : AA + g + G]
                web = stats[:, WE + g : WE + g + G]
                w1b = stats[:, W1 + g : W1 + g + G]
                # z1u = (delta + cdiff) + u0
                nc.vector.scalar_tensor_tensor(
                    out=z1b, in0=dlb, scalar=cdc, in1=u0b,
                    op0=ALU.add, op1=ALU.add,
                )
                # a = max(z1u, u0)
                nc.vector.tensor_tensor(out=aab, in0=z1b, in1=u0b, op=ALU.max)
                nc.scalar.activation(out=web, in_=aab, func=ACT.Exp)
                nc.scalar.copy(webf[:, g : g + G], web)
                # w1 = sigmoid(delta + cdiff)
                nc.scalar.activation(out=w1b, in_=dlb, func=ACT.Sigmoid, bias=cdc)

                # per-tile: PE outer products + comp2 + output_two accum
                for t in range(g, g + G):
                    in_t = in_ts[t]
                    # stationary [2,128] = transpose of the (1, w1_t) pair
                    wst_ps = wstpsp.tile([2, 128], f32, tag="wst")
                    nc.tensor.transpose(wst_ps, pair_view[:, t, :], identity)
                    nc.scalar.copy(est[:, ts(t, 128)], wst_ps)

                    # output_one = 1*mem0 + w1*(mem1-mem0)
                    o1_ps = o1psp.tile([128, D], f32, tag="o1")
                    for h in range(2):
                        nc.tensor.matmul(
                            o1_ps[:, ts(h, 512)],
                            lhsT=est[:, ts(t, 128)],
                            rhs=m0db[:, ts(h, 512)],
                            start=True,
                            stop=True,
                        )
                    st12 = stage12p.tile([128, 2 * D], f32, tag="s12")
                    nc.scalar.activation(out=st12[:, 0:D], in_=o1_ps, func=ACT.Copy)
                    # comp2 = input * output_one
                    nc.vector.tensor_tensor(
                        out=st12[:, D : 2 * D], in0=in_t, in1=o1_ps, op=ALU.mult
                    )
                    nc.scalar.dma_start(out=out[ts(t, 128), D : 3 * D], in_=st12)
                    # output_two partials: o2_ps += wexp_t^T @ in_t (PE accum)
                    in_bf = in_bfs[t]
                    for h in range(2):
                        nc.tensor.matmul(
                            o2_ps[0:1, ts(h, 512)],
                            lhsT=webf[:, t : t + 1],
                            rhs=in_bf[:, ts(h, 512)],
                            start=(t == 0),
                            stop=(t == T - 1),
                        )

            # ---------------- output_two normalize + q rows ----------------
            nc.vector.tensor_reduce(
                out=stats[:, SE : SE + 1], in_=stats[:, WE : WE + T],
                axis=mybir.AxisListType.X, op=ALU.add,
            )
            stot_ps = wstpsp.tile([1, 1], f32, tag="wst")
            nc.tensor.matmul(
                stot_ps, lhsT=stats[:, SE : SE + 1], rhs=stats[:, OC : OC + 1],
                start=True, stop=True,
            )
            nc.scalar.copy(stats[0:1, ST : ST + 1], stot_ps)
            nc.vector.reciprocal(stats[0:1, SR : SR + 1], stats[0:1, ST : ST + 1])

            # o2n = output_two (normalized) on partition 0
            o2n_sb = setup1d.tile([1, D], f32, tag="s1d")
            nc.scalar.activation(
                out=o2n_sb, in_=o2_ps, func=ACT.Copy,
                scale=stats[0:1, SR : SR + 1],
            )
            # broadcast o2n to 2 partitions, qcat = o2n * (mem0, memd)
            o2nc = setup2d.tile([2, D], f32, tag="s2d")
            for h in range(2):
                q_ps = wstpsp.tile([2, 512], f32, tag="wst")
                nc.tensor.matmul(
                    q_ps, lhsT=ones2, rhs=o2n_sb[:, ts(h, 512)], start=True, stop=True
                )
                nc.scalar.copy(o2nc[:, ts(h, 512)], q_ps)
            qcat = consts.tile([2, D], bf16)
            nc.vector.tensor_tensor(out=qcat, in0=o2nc, in1=m0d, op=ALU.mult)

            # ---------------- comp3 pass (PE) --------------------------
            for t in range(T):
                c3_ps = o1psp.tile([128, D], f32, tag="o1")
                for h in range(2):
                    nc.tensor.matmul(
                        c3_ps[:, ts(h, 512)],
                        lhsT=est[:, ts(t, 128)],
                        rhs=qcat[:, ts(h, 512)],
                        start=True,
                        stop=True,
                    )
                st3 = stage3p.tile([128, D], f32, tag="s3")
                nc.scalar.copy(st3, c3_ps)
                nc.gpsimd.dma_start(out=out[ts(t, 128), 3 * D : 4 * D], in_=st3)

    nc.compile()
    return nc


def _get_nc():
    if "nc" not in _CACHE:
        _CACHE["nc"] = _build()
    return _CACHE["nc"]


def kernel(input, memory, w_input, w_memory, dot_scale):
    from concourse.bass_utils import run_bass_kernel_spmd

    nc = _get_nc()
    input = np.ascontiguousarray(input, dtype=np.float32)
    memory = np.ascontiguousarray(memory, dtype=np.float32)
    w_input = np.ascontiguousarray(w_input, dtype=np.float32)
    w_memory = np.ascontiguousarray(w_memory, dtype=np.float32)
    dot_scale = np.ascontiguousarray(dot_scale, dtype=np.float32)
    in_maps = [
        {
            "input": input[b],
            "memory": memory[b],
            "w_input": w_input,
            "w_memory": w_memory,
            "dot_scale": dot_scale,
        }
        for b in range(B)
    ]
    res = run_bass_kernel_spmd(nc, in_maps, core_ids=list(range(B)))
    return np.stack([res.results[b]["out"] for b in range(B)], axis=0)


# revision 10
# speedup vs baseline: 1.3937x; 1.1690x over previous
"""BiDAF-style co-attention (memory_len=2) Trainium2 Bass kernel.

Full inputs:
  input     [8, 4096, 1024] f32
  memory    [8, 2, 1024]    f32
  w_input   [1024] f32, w_memory [1024] f32, dot_scale [1024] f32
Output:
  concat([input, output_one, input*output_one, output_two*output_one], -1)
  -> [8, 4096, 4096] f32

Sharding: data-parallel over batch; core b gets batch b (8 cores).

Math (per batch), 2-way softmax as a sigmoid:
  v_m   = w_input + dot_scale * mem_m            (d-vector, m=0,1)
  c_m   = mem_m . w_memory                       (scalar)
  delta[l] = input[l].(v1-v0);  u0[l] = input[l].v0   (two DVE dot passes)
  w1[l] = sigmoid(delta + cdiff)                 (ACT, bias=cdiff)
  output_one[l] = mem0 + w1[l]*(mem1-mem0)       (PE rank-2, stationary (1,w1),
                                                  rhs rows (mem0, mem1-mem0))
  a[l]  = max(u0, u0+delta+cdiff); wexp = exp(a)
  output_two = (sum_l wexp[l]*input[l]) / sum wexp   (PE matvec, PSUM accum)
  comp3[l] = o2n*output_one[l] = q0 + w1[l]*qd   (PE, same stationary,
                                                  rhs rows (o2n*mem0, o2n*memd))
PE matmuls run in bf16 (1 cyc/row); setup/broadcasts stay fp32 exact.
"""

import numpy as np

B, L, D = 8, 4096, 1024
T = L // 128  # 32 row-tiles of 128
G = 8         # tiles per group (batched small stats ops)

_CACHE = {}

# stats column layout ([128, NSTAT] f32), blocks of 32 (col t = tile t)
DL = 0       # delta = in . (v1-v0)
U0 = 32      # u0 = in . v0
Z1 = 64      # z1u = delta + cdiff + u0
AA = 96      # a = max(z1u, u0)
WE = 128     # wexp = exp(a)
W0 = 160     # all-ones block (memset)  -> stationary pair (W0+t, W1+t)
W1 = 192     # w1 = sigmoid(delta + cdiff)
OC = 224     # ones column
CD, SE, ST, SR = 225, 226, 227, 228
NSTAT = 232


def _build():
    import concourse.bacc as bacc
    import concourse.bass as bass
    import concourse.tile as tile
    from concourse import mybir
    from concourse.masks import make_identity

    f32 = mybir.dt.float32
    bf16 = mybir.dt.bfloat16
    ALU = mybir.AluOpType
    ACT = mybir.ActivationFunctionType

    nc = bacc.Bacc("TRN2", target_bir_lowering=False, debug=False)

    inp = nc.dram_tensor("input", [L, D], f32, kind="ExternalInput").ap()
    mem = nc.dram_tensor("memory", [2, D], f32, kind="ExternalInput").ap()
    w_in = nc.dram_tensor("w_input", [D], f32, kind="ExternalInput").ap()
    w_mem = nc.dram_tensor("w_memory", [D], f32, kind="ExternalInput").ap()
    d_sc = nc.dram_tensor("dot_scale", [D], f32, kind="ExternalInput").ap()
    out = nc.dram_tensor("out", [L, 4 * D], f32, kind="ExternalOutput").ap()

    def bc(src_ap, n_part, n_free):
        # broadcast-read AP: n_part partitions each reading the same n_free
        # contiguous elements at src_ap's offset (DMA-only pattern)
        return bass.AP(src_ap.tensor, src_ap.offset, [[0, n_part], [1, n_free]])

    ts = bass.ts

    with tile.TileContext(nc) as tc:
        with (
            tc.tile_pool(name="consts", bufs=1) as consts,
            tc.tile_pool(name="setup2d", bufs=3) as setup2d,
            tc.tile_pool(name="setup1d", bufs=2) as setup1d,
            tc.tile_pool(name="inp_pool", bufs=6) as inp_pool,
            tc.tile_pool(name="inbf_pool", bufs=18) as inbf_pool,
            tc.tile_pool(name="scratch", bufs=2) as scratch,
            tc.tile_pool(name="stage12", bufs=4) as stage12p,
            tc.tile_pool(name="stage3", bufs=6) as stage3p,
            tc.tile_pool(name="o1ps", bufs=2, space="PSUM") as o1psp,
            tc.tile_pool(name="o2ps", bufs=1, space="PSUM") as o2psp,
            tc.tile_pool(name="wstps", bufs=2, space="PSUM") as wstpsp,
        ):
            # ---------------- setup ----------------
            mem_sb = consts.tile([2, D], f32)
            nc.sync.dma_start(out=mem_sb, in_=mem)
            stats = consts.tile([128, NSTAT], f32)
            identity = consts.tile([128, 128], f32)
            make_identity(nc, identity)
            nc.vector.memset(stats[:, OC : OC + 1], 1.0)
            nc.vector.memset(stats[:, W0 : W0 + T], 1.0)
            # est: per-tile transposed (1, w1) stationaries, col-block t
            est = consts.tile([2, T * 128], bf16)
            # strided pair view: pair_view[:, t, :] = cols (W0+t, W1+t)
            pair_view = stats[:, W0 : W0 + 2 * T].rearrange("p (a b) -> p b a", a=2)

            # small constant stationaries (engine ops must start at partition
            # 0, so build 2-partition constants from identity slices)
            p10 = identity[0:2, 0:1]            # col (x0) = [1, 0]
            pm1 = consts.tile([2, 1], f32)      # col (x1 - x0) = [-1, 1]
            nc.vector.tensor_tensor(
                out=pm1, in0=identity[0:2, 1:2], in1=identity[0:2, 0:1],
                op=ALU.subtract,
            )
            w22 = consts.tile([2, 2], f32)      # [[1,-1],[0,1]]: cols (x0, x1-x0)
            nc.vector.tensor_copy(w22, identity[0:2, 0:2])
            nc.vector.tensor_tensor(
                out=w22[:, 1:2], in0=w22[:, 1:2], in1=identity[0:2, 0:1],
                op=ALU.subtract,
            )
            ones1 = consts.tile([1, 128], f32)  # broadcast to 128 partitions
            nc.vector.memset(ones1, 1.0)
            ones2 = consts.tile([1, 2], f32)    # broadcast to 2 partitions
            nc.vector.memset(ones2, 1.0)

            ds_b = setup2d.tile([2, D], f32, tag="s2d")
            nc.sync.dma_start(out=ds_b, in_=bc(d_sc, 2, D))
            win_b = setup2d.tile([2, D], f32, tag="s2d")
            nc.sync.dma_start(out=win_b, in_=bc(w_in, 2, D))
            # vcat = mem*ds + w_in  (rows: v0, v1)
            vcat = setup2d.tile([2, D], f32, tag="s2d")
            nc.vector.tensor_tensor(out=vcat, in0=mem_sb, in1=ds_b, op=ALU.mult)
            nc.vector.tensor_tensor(out=vcat, in0=vcat, in1=win_b, op=ALU.add)

            # m0d = (mem0, mem1-mem0) rows via PE; v0/vdiff as partition-0 rows
            m0d = consts.tile([2, D], f32)
            m0db = consts.tile([2, D], bf16)
            v0row = setup1d.tile([1, D], f32, tag="s1d")
            vdrow = setup1d.tile([1, D], f32, tag="s1d")
            for h in range(2):
                md_ps = wstpsp.tile([2, 512], f32, tag="wst")
                nc.tensor.matmul(
                    md_ps, lhsT=w22, rhs=mem_sb[:, ts(h, 512)], start=True, stop=True
                )
                nc.scalar.copy(m0d[:, ts(h, 512)], md_ps)
                nc.scalar.copy(m0db[:, ts(h, 512)], md_ps)
                v0_ps = wstpsp.tile([1, 512], f32, tag="wst")
                nc.tensor.matmul(
                    v0_ps, lhsT=p10, rhs=vcat[:, ts(h, 512)], start=True, stop=True
                )
                nc.scalar.copy(v0row[:, ts(h, 512)], v0_ps)
                vd_ps = wstpsp.tile([1, 512], f32, tag="wst")
                nc.tensor.matmul(
                    vd_ps, lhsT=pm1, rhs=vcat[:, ts(h, 512)], start=True, stop=True
                )
                nc.scalar.copy(vdrow[:, ts(h, 512)], vd_ps)

            # broadcast v0 / vdiff rows to 128 partitions via PE (exact fp32)
            v0b = consts.tile([128, D], bf16)
            vdb = consts.tile([128, D], bf16)
            for h in range(2):
                b_ps = o1psp.tile([128, 512], f32, tag="o1")
                nc.tensor.matmul(
                    b_ps, lhsT=ones1, rhs=v0row[:, ts(h, 512)], start=True, stop=True
                )
                nc.scalar.copy(v0b[:, ts(h, 512)], b_ps)
                b_ps2 = o1psp.tile([128, 512], f32, tag="o1")
                nc.tensor.matmul(
                    b_ps2, lhsT=ones1, rhs=vdrow[:, ts(h, 512)], start=True, stop=True
                )
                nc.scalar.copy(vdb[:, ts(h, 512)], b_ps2)

            # memdot = (mem * w_memory).sum(-1) -> [2,1]; cdc = bcast(c1-c0)
            wmem_b = setup2d.tile([2, D], f32, tag="s2d")
            nc.sync.dma_start(out=wmem_b, in_=bc(w_mem, 2, D))
            sc2 = setup2d.tile([2, D], f32, tag="s2d")
            md_col = setup1d.tile([2, 1], f32, tag="s1s")
            nc.vector.scalar_tensor_tensor(
                out=sc2, in0=mem_sb, scalar=1.0, in1=wmem_b,
                op0=ALU.mult, op1=ALU.mult,
                accum_out=md_col,
            )
            cd_ps = wstpsp.tile([1, 1], f32, tag="wst")
            nc.tensor.matmul(cd_ps, lhsT=pm1, rhs=md_col, start=True, stop=True)
            cd_sb = setup1d.tile([1, 1], f32, tag="s1s")
            nc.scalar.copy(cd_sb, cd_ps)
            cdc_ps = wstpsp.tile([128, 1], f32, tag="wst")
            nc.tensor.matmul(cdc_ps, lhsT=ones1, rhs=cd_sb, start=True, stop=True)
            nc.scalar.copy(stats[:, CD : CD + 1], cdc_ps)

            webf = consts.tile([128, T], bf16)

            cdc = stats[:, CD : CD + 1]

            # persistent PSUM accumulator for output_two partials [1, D]
            o2_ps = o2psp.tile([1, D], f32)

            # ---------------- main pass ----------------
            for g in range(0, T, G):
                in_bfs = {}
                # per-tile: load + two fused att dots (DVE)
                for t in range(g, g + G):
                    in_t = inp_pool.tile([128, D], f32, tag="in_t")
                    nc.sync.dma_start(out=in_t, in_=inp[ts(t, 128), :])
                    # comp0: passthrough copy of input
                    nc.gpsimd.dma_start(out=out[ts(t, 128), 0:D], in_=in_t)
                    in_bf = inbf_pool.tile([128, D], bf16, tag="in_bf")
                    in_bfs[t] = in_bf
                    nc.vector.tensor_copy(in_bf, in_t)
                    sc_t = scratch.tile([128, D], bf16, tag="ttr")
                    nc.vector.scalar_tensor_tensor(
                        out=sc_t, in0=in_bf, scalar=1.0, in1=vdb,
                        op0=ALU.mult, op1=ALU.mult,
                        accum_out=stats[:, DL + t : DL + t + 1],
                    )
                    sc_t2 = scratch.tile([128, D], bf16, tag="ttr")
                    nc.vector.scalar_tensor_tensor(
                        out=sc_t2, in0=in_bf, scalar=1.0, in1=v0b,
                        op0=ALU.mult, op1=ALU.mult,
                        accum_out=stats[:, U0 + t : U0 + t + 1],
                    )

                # batched group stats ([128, G] blocks)
                dlb = stats[:, DL + g : DL + g + G]
                u0b = stats[:, U0 + g : U0 + g + G]
                z1b = stats[:, Z1 + g : Z1 + g + G]
                aab = stats[:, AA + g : AA + g + G]
                web = stats[:, WE + g : WE + g + G]
                w1b = stats[:, W1 + g : W1 + g + G]
                # z1u = (delta + cdiff) + u0
                nc.vector.scalar_tensor_tensor(
                    out=z1b, in0=dlb, scalar=cdc, in1=u0b,
                    op0=ALU.add, op1=ALU.add,
                )
                # a = max(z1u, u0)
                nc.vector.tensor_tensor(out=aab, in0=z1b, in1=u0b, op=ALU.max)
                nc.scalar.activation(out=web, in_=aab, func=ACT.Exp)
                nc.scalar.copy(webf[:, g : g + G], web)
                # w1 = sigmoid(delta + cdiff)
                nc.scalar.activation(out=w1b, in_=dlb, func=ACT.Sigmoid, bias=cdc)

                # per-tile: PE outer products + comp2 + output_two accum
                for t in range(g, g + G):
                    in_bf = in_bfs[t]
                    # stationary [2,128] = transpose of the (1, w1_t) pair
                    wst_ps = wstpsp.tile([2, 128], f32, tag="wst")
                    nc.tensor.transpose(wst_ps, pair_view[:, t, :], identity)
                    nc.scalar.copy(est[:, ts(t, 128)], wst_ps)

                    # output_one = 1*mem0 + w1*(mem1-mem0)
                    o1_ps = o1psp.tile([128, D], f32, tag="o1")
                    for h in range(2):
                        nc.tensor.matmul(
                            o1_ps[:, ts(h, 512)],
                            lhsT=est[:, ts(t, 128)],
                            rhs=m0db[:, ts(h, 512)],
                            start=True,
                            stop=True,
                        )
                    st12 = stage12p.tile([128, 2 * D], f32, tag="s12")
                    nc.scalar.activation(out=st12[:, 0:D], in_=o1_ps, func=ACT.Copy)
                    # comp2 = input * output_one
                    nc.vector.tensor_tensor(
                        out=st12[:, D : 2 * D], in0=in_bf, in1=o1_ps, op=ALU.mult
                    )
                    nc.scalar.dma_start(out=out[ts(t, 128), D : 3 * D], in_=st12)
                    # output_two partials: o2_ps += wexp_t^T @ in_bf (PE accum)
                    for h in range(2):
                        nc.tensor.matmul(
                            o2_ps[0:1, ts(h, 512)],
                            lhsT=webf[:, t : t + 1],
                            rhs=in_bf[:, ts(h, 512)],
                            start=(t == 0),
                            stop=(t == T - 1),
                        )

            # ---------------- output_two normalize + q rows ----------------
            nc.vector.tensor_reduce(
                out=stats[:, SE : SE + 1], in_=stats[:, WE : WE + T],
                axis=mybir.AxisListType.X, op=ALU.add,
            )
            stot_ps = wstpsp.tile([1, 1], f32, tag="wst")
            nc.tensor.matmul(
                stot_ps, lhsT=stats[:, SE : SE + 1], rhs=stats[:, OC : OC + 1],
                start=True, stop=True,
            )
            nc.scalar.copy(stats[0:1, ST : ST + 1], stot_ps)
            nc.vector.reciprocal(stats[0:1, SR : SR + 1], stats[0:1, ST : ST + 1])

            # o2n = output_two (normalized) on partition 0
            o2n_sb = setup1d.tile([1, D], f32, tag="s1d")
            nc.scalar.activation(
                out=o2n_sb, in_=o2_ps, func=ACT.Copy,
                scale=stats[0:1, SR : SR + 1],
            )
            # broadcast o2n to 2 partitions, qcat = o2n * (mem0, memd)
            o2nc = setup2d.tile([2, D], f32, tag="s2d")
            for h in range(2):
                q_ps = wstpsp.tile([2, 512], f32, tag="wst")
                nc.tensor.matmul(
                    q_ps, lhsT=ones2, rhs=o2n_sb[:, ts(h, 512)], start=True, stop=True
                )
                nc.scalar.copy(o2nc[:, ts(h, 512)], q_ps)
            qcat = consts.tile([2, D], bf16)
            nc.vector.tensor_tensor(out=qcat, in0=o2nc, in1=m0d, op=ALU.mult)

            # ---------------- comp3 pass (PE) --------------------------
            for t in range(T):
                c3_ps = o1psp.tile([128, D], f32, tag="o1")
                for h in range(2):
                    nc.tensor.matmul(
                        c3_ps[:, ts(h, 512)],
                        lhsT=est[:, ts(t, 128)],
                        rhs=qcat[:, ts(h, 512)],
                        start=True,
                        stop=True,
                    )
                st3 = stage3p.tile([128, D], f32, tag="s3")
                nc.scalar.copy(st3, c3_ps)
                nc.gpsimd.dma_start(out=out[ts(t, 128), 3 * D : 4 * D], in_=st3)

    nc.compile()
    return nc


def _get_nc():
    if "nc" not in _CACHE:
        _CACHE["nc"] = _build()
    return _CACHE["nc"]


def kernel(input, memory, w_input, w_memory, dot_scale):
    from concourse.bass_utils import run_bass_kernel_spmd

    nc = _get_nc()
    input = np.ascontiguousarray(input, dtype=np.float32)
    memory = np.ascontiguousarray(memory, dtype=np.float32)
    w_input = np.ascontiguousarray(w_input, dtype=np.float32)
    w_memory = np.ascontiguousarray(w_memory, dtype=np.float32)
    dot_scale = np.ascontiguousarray(dot_scale, dtype=np.float32)
    in_maps = [
        {
            "input": input[b],
            "memory": memory[b],
            "w_input": w_input,
            "w_memory": w_memory,
            "dot_scale": dot_scale,
        }
        for b in range(B)
    ]
    res = run_bass_kernel_spmd(nc, in_maps, core_ids=list(range(B)))
    return np.stack([res.results[b]["out"] for b in range(B)], axis=0)
